# revision 4
# baseline (speedup 1.0000x reference)
"""Hard-negative contrastive loss on 8 TRN2 NeuronCores (Bass/Tile).

Reference semantics (B=1024, Q=32, D=512, temp scalar):
    sim[i,j,q] = fusion[i] . target[j,q];  v[i,j] = max_q sim / temp
    loss = mean_i(lse_j(v[i,:]) - v[i,i])
         + 0.5 * mean_i(log(exp(pos) + sum exp(top512 offdiag)) - pos)

v2 design:
  - Inputs shipped as float8_e3m4 (scaled by 16 per operand; loss error
    ~5e-5 rel, tolerance 2e-2). fusT [128,4,1024] + tgtT [128,4,4096]
    per core = 2.5 MiB (vs 10.5 MiB f32) -- cuts per-exec staging.
  - Target rows j split 128/core. Each core computes its (1024 x 128)
    column block of raw products r = 256*sim via fp8 matmuls (contraction
    d on partitions), reduce_max over q -> bf16 P_sb. AllToAll (bf16,
    split in two j-halves so the first overlaps the remaining matmuls)
    swaps blocks so core c holds full rows c*128..c*128+127.
  - Diagonal handled with a runtime partition_id dynamic slice (no
    onehot input): pos extracted from the diag block, then the block
    gets -1e9 punched in-place before top-512 selection.
  - Per-row top-512 threshold by 13-step bisection on the raw bf16 row
    (count via fused is_gt + accumulate); entries between the final
    bounds are approximated at the upper bound (error << tolerance).
  - Host averages the 1024 per-row (std, hard) losses.
"""
import sys

if "/opt/trn_rl_repo" not in sys.path:
    sys.path.insert(0, "/opt/trn_rl_repo")

import numpy as np
import ml_dtypes

N_CORES = 8
B, Q, D = 1024, 32, 512
KC = D // 128                  # 4 contraction chunks
JQ = (B // N_CORES) * Q        # 4096 target vectors per core
NBLK = 512                     # jq per matmul / psum tile
JBLK = NBLK // Q               # 16 j columns per psum tile
N_ITERS = 13                   # bisection steps
NUM_HARD = B // 2              # 512
S = 16.0                       # per-operand fp8 scale
BASE_TEMP = 0.07               # folded into host-side fusion scale
VS = 1.0 / (S * S * BASE_TEMP)  # raw-product -> v units
NEGB = -1.0e9

_RUNNER = None


def _build():
    import concourse.bacc as bacc
    import concourse.bass as bass
    import concourse.mybir as mybir
    import concourse.tile as tile

    f32 = mybir.dt.float32
    bf16 = mybir.dt.bfloat16
    f8 = mybir.dt.float8e3
    i32 = mybir.dt.int32
    Alu = mybir.AluOpType
    Act = mybir.ActivationFunctionType
    X = mybir.AxisListType.X

    nc = bacc.Bacc(None, target_bir_lowering=False, debug=False,
                   num_devices=N_CORES)

    fusT_ap = nc.dram_tensor("fusT", [128, KC, B], f8, kind="ExternalInput").ap()
    tgtT_ap = nc.dram_tensor("tgtT", [128, KC, JQ], f8, kind="ExternalInput").ap()
    out_ap = nc.dram_tensor("rowloss", [128, 2], f32, kind="ExternalOutput").ap()

    with tile.TileContext(nc) as tc:
        with (
            tc.tile_pool(name="fus", bufs=1) as fus_pool,
            tc.tile_pool(name="tgt", bufs=1) as tgt_pool,
            tc.tile_pool(name="res", bufs=1) as res_pool,
            tc.tile_pool(name="big", bufs=1) as big_pool,
            tc.tile_pool(name="small", bufs=1) as small_pool,
            tc.tile_pool(name="psum", bufs=8, space="PSUM") as psum_pool,
            tc.tile_pool(name="dram", bufs=1, space="DRAM") as dram_pool,
        ):
            pid = nc.vector.partition_id()

            # diag punch block: -1e9 on the diagonal, 0 elsewhere
            NEGD = res_pool.tile([128, 128], bf16)
            tmp128 = res_pool.tile([128, 128], bf16)
            nc.gpsimd.memset(tmp128[:], NEGB)
            nc.gpsimd.affine_select(NEGD[:], tmp128[:], [[1, 128]],
                                    Alu.is_equal, 0.0, base=0,
                                    channel_multiplier=-1)

            # ---------- load fp8 operands ----------
            fus = fus_pool.tile([128, KC, B], f8)
            nc.sync.dma_start(fus[:], fusT_ap[:])
            tgt = tgt_pool.tile([128, KC, JQ], f8)
            for k in range(KC):
                nc.sync.dma_start(tgt[:, k, :], tgtT_ap[:, k, :])

            # ---------- phase 1: my (1024 x 128) block, max over q ----------
            P_sb = res_pool.tile([128, N_CORES, 128], bf16)  # [i_part, it, j]
            p_in1 = dram_pool.tile([B, 64], bf16)
            p_out1 = dram_pool.tile([B, 64], bf16)
            p_in2 = dram_pool.tile([B, 64], bf16)
            p_out2 = dram_pool.tile([B, 64], bf16)

            for b in range(JQ // NBLK):
                for it in range(N_CORES):
                    ps = psum_pool.tile([128, NBLK], f32)
                    for k in range(KC):
                        nc.tensor.matmul(
                            ps[:],
                            fus[:, k, it * 128:(it + 1) * 128],
                            tgt[:, k, b * NBLK:(b + 1) * NBLK],
                            start=(k == 0),
                            stop=(k == KC - 1),
                        )
                    nc.vector.reduce_max(
                        P_sb[:, it, b * JBLK:(b + 1) * JBLK],
                        ps.rearrange("p (j q) -> p j q", q=Q),
                        axis=X,
                    )
                if b == 3:
                    # first j-half ready for all i-tiles: overlap A2A #1
                    for it in range(N_CORES):
                        nc.sync.dma_start(p_in1[it * 128:(it + 1) * 128, :],
                                          P_sb[:, it, 0:64])
                    nc.gpsimd.collective_compute(
                        "AllToAll", Alu.bypass,
                        replica_groups=[list(range(N_CORES))],
                        ins=[p_in1.opt()], outs=[p_out1.opt()])

            for it in range(N_CORES):
                nc.sync.dma_start(p_in2[it * 128:(it + 1) * 128, :],
                                  P_sb[:, it, 64:128])
            nc.gpsimd.collective_compute(
                "AllToAll", Alu.bypass,
                replica_groups=[list(range(N_CORES))],
                ins=[p_in2.opt()], outs=[p_out2.opt()])

            V = big_pool.tile([128, B], bf16)
            for s in range(N_CORES):
                nc.sync.dma_start(V[:, s * 128:s * 128 + 64],
                                  p_out1[s * 128:(s + 1) * 128, :])
                nc.sync.dma_start(V[:, s * 128 + 64:(s + 1) * 128],
                                  p_out2[s * 128:(s + 1) * 128, :])

            # ---------- phase 2: per-row losses (raw product units) ----------
            E = big_pool.tile([128, B], bf16)
            junkb = big_pool.tile([128, B], bf16)
            Dblk = res_pool.tile([128, 128], bf16)
            Dsel = res_pool.tile([128, 128], bf16)

            def sm(name, dt=f32):
                return small_pool.tile([128, 1], dt, name=name, tag=name)

            m, negm_s, lo, hi, mid, cnt, cnt_hi = (
                sm(n) for n in "m negm_s lo hi mid cnt cnt_hi".split())
            pos, sumfull, sumsel, ehi, epos, rem, acc = (
                sm(n) for n in "pos sumfull sumsel ehi epos rem acc".split())
            lnf, lnh, t0 = sm("lnf"), sm("lnh"), sm("t0")
            upd = sm("upd", i32)
            updn = sm("updn", i32)

            nc.vector.reduce_max(m[:], V[:], axis=X)
            nc.vector.tensor_reduce(lo[:], V[:], axis=X, op=Alu.min)
            nc.vector.tensor_scalar_add(lo[:], lo[:], -1.0)
            nc.vector.tensor_scalar_mul(negm_s[:], m[:], -VS)

            # E = exp(VS*V - VS*m) over the full row (incl diag), sum -> sumfull
            nc.scalar.activation(E[:], V[:], Act.Exp, bias=negm_s[:], scale=VS,
                                 accum_out=sumfull[:])

            # diag block: extract pos, then punch -1e9 in place
            dslice = bass.ts(pid, 128)
            nc.vector.tensor_copy(Dblk[:], V[:, dslice])
            nc.gpsimd.affine_select(Dsel[:], Dblk[:], [[1, 128]],
                                    Alu.is_equal, NEGB, base=0,
                                    channel_multiplier=-1)
            nc.vector.reduce_max(pos[:], Dsel[:], axis=X)
            nc.vector.tensor_add(V[:, dslice], V[:, dslice], NEGD[:])

            nc.vector.tensor_copy(hi[:], m[:])
            for _ in range(N_ITERS):
                nc.vector.tensor_add(mid[:], lo[:], hi[:])
                nc.vector.tensor_scalar_mul(mid[:], mid[:], 0.5)
                nc.vector.tensor_scalar(
                    junkb[:], V[:], mid[:], None, op0=Alu.is_gt,
                    op1=Alu.add, accum_out=cnt[:])
                nc.vector.tensor_scalar(upd[:], cnt[:], float(NUM_HARD), None,
                                        op0=Alu.is_gt)
                nc.vector.tensor_scalar(updn[:], cnt[:], float(NUM_HARD), None,
                                        op0=Alu.is_le)
                nc.vector.copy_predicated(lo[:], upd[:], mid[:])
                nc.vector.copy_predicated(hi[:], updn[:], mid[:])

            # count and exp-sum of the confirmed top entries (v > hi)
            nc.vector.tensor_scalar(
                junkb[:], V[:], hi[:], None, op0=Alu.is_gt,
                op1=Alu.add, accum_out=cnt_hi[:])
            nc.vector.scalar_tensor_tensor(
                junkb[:], V[:], hi[:], E[:], op0=Alu.is_gt, op1=Alu.mult,
                accum_out=sumsel[:])

            nc.scalar.activation(epos[:], pos[:], Act.Exp, bias=negm_s[:],
                                 scale=VS)
            nc.scalar.activation(ehi[:], hi[:], Act.Exp, bias=negm_s[:],
                                 scale=VS)
            # acc = epos + sumsel + (512 - cnt_hi) * ehi
            nc.vector.tensor_scalar(rem[:], cnt_hi[:], -1.0, float(NUM_HARD),
                                    op0=Alu.mult, op1=Alu.add)
            nc.vector.tensor_mul(rem[:], rem[:], ehi[:])
            nc.vector.tensor_add(acc[:], epos[:], sumsel[:])
            nc.vector.tensor_add(acc[:], acc[:], rem[:])

            outs = res_pool.tile([128, 2], f32)
            # loss_std = VS*(m-pos) + ln(sumfull); loss_hard = VS*(m-pos) + ln(acc)
            nc.scalar.activation(lnf[:], sumfull[:], Act.Ln)
            nc.scalar.activation(lnh[:], acc[:], Act.Ln)
            nc.vector.tensor_sub(t0[:], m[:], pos[:])
            nc.vector.tensor_scalar_mul(t0[:], t0[:], VS)
            nc.vector.tensor_add(outs[:, 0:1], t0[:], lnf[:])
            nc.vector.tensor_add(outs[:, 1:2], t0[:], lnh[:])

            nc.sync.dma_start(out_ap[:], outs[:])

    nc.compile()
    return nc


def _get_nc():
    global _RUNNER
    if _RUNNER is None:
        _RUNNER = _build()
    return _RUNNER


def make_in_maps(fusion_feats, target_feats, temp):
    fusion = np.asarray(fusion_feats, dtype=np.float32)
    target = np.asarray(target_feats, dtype=np.float32)
    t = float(np.asarray(temp))
    f8 = ml_dtypes.float8_e3m4
    sf = S * (BASE_TEMP / t)   # keeps compile-time VS valid for any temp
    fusT = np.ascontiguousarray(
        (fusion * sf).T.reshape(KC, 128, B).transpose(1, 0, 2)).astype(f8)
    rows_per = B // N_CORES
    in_maps = []
    for c in range(N_CORES):
        shard = (target[c * rows_per:(c + 1) * rows_per] * S).reshape(JQ, D)
        tgtT = np.ascontiguousarray(
            shard.T.reshape(KC, 128, JQ).transpose(1, 0, 2)).astype(f8)
        in_maps.append({"fusT": fusT, "tgtT": tgtT})
    return in_maps


def combine(results):
    rows = np.concatenate([r["rowloss"] for r in results], axis=0)  # (1024, 2)
    loss = rows[:, 0].mean(dtype=np.float32) \
        + np.float32(0.5) * rows[:, 1].mean(dtype=np.float32)
    return np.asarray(loss, dtype=np.float32)


def kernel(fusion_feats, target_feats, temp):
    from concourse import bass_utils

    nc = _get_nc()
    in_maps = make_in_maps(fusion_feats, target_feats, temp)
    res = bass_utils.run_bass_kernel_spmd(nc, in_maps, list(range(N_CORES)))
    return combine(res.results)
